# revision 1
# baseline (speedup 1.0000x reference)
"""Trainium2 Bass kernel for nn_CustomLoss (CrossEntropy + binary-remap BCE).

loss = mean_i[ logsumexp(pred_i) - pred_i[t_i] ]
     + 100 * mean_i[ 1{ LUT[argmax(pred_i)] != LUT[t_i] } ]

with LUT = [0,0,1,1,1,1,1,1,0,0]  (LUT[j] = 1 iff 2 <= j <= 7).

Sharding: data-parallel over the batch axis across 8 NeuronCores; each core
returns 3 per-partition partial sums which the host folds into the scalar.

The DVE's second SBUF read port is shared with GPSIMD, so 2-port DVE
instructions and GPSIMD instructions destroy each other's throughput.  This
version keeps every DVE instruction single-ported:

  * The host packs each row as 11 floats: [h, pred0+16 .. pred9+16] where
    h = 1000 + 11*w + 1 + t + 0.25*bt  (t = target class, bt = binary
    target).  h dominates every shifted logit and increases with w, so a
    single running-max scan inside the fused gather op recovers the
    current row's target position with no second tensor stream.
  * GATHER_SCAN_ANT (DVE, 1 port): hold = scanmax(x); select x where
    hold - (1000 + k) in [0, 0.5); accumulate  ->  sum of pred'[t].
  * SB_EXTRACT_ANT (DVE, 1 port): sb = 1 - 2*(frac(h) > 0) in {+1,-1}
    from the 0.25*bt fraction, via the +2^23 round trick.
  * mid-6/outer-4 maxes: strided tensor_reduce (1 port), the outer-4 with
    negate=True so GPSIMD (add/mult only) forms d6 = m6 - m4 and
    q = d6 * sb.
  * SUM_GT_ANT (DVE, 1 port): accumulate (q > 0) = binary mismatches.

  DMA   : packed tiles [128, W*11] f32 (contiguous rows)
  ACT   : E = exp(pred' - 16) -> bf16 on the strided logit columns (combined
          Exp/Ln table), Ln of row-sums with per-partition accumulate
  GPSIMD: per-row sum of E via bf16 add tree (10 -> 5 -> 2+1 -> 1), d6, q
  DVE   : the reduces + three single-ported fused ops above
"""

import numpy as np

# ---------------------------------------------------------------- constants
N = 2_000_000
C = 10
CW = 11                       # row width incl. the packed sentinel column
N_CORES = 8
P = 128
# variable tile widths: small first tile to start compute early, small last
# tile to shorten the drain; sum * P = padded rows per core
TILE_WS = [123, 489, 489, 489, 366]
W_SUM = sum(TILE_WS)          # 1,956
ROWS_CORE_PAD = P * W_SUM     # 250,368
ROWS_CORE = N // N_CORES      # 250,000
PAD_PER_CORE = ROWS_CORE_PAD - ROWS_CORE  # 368
SHIFT = 16.0
HBASE = 1000.0

_CACHE = {}


# ------------------------------------------------------- custom DVE ops
def _register_custom_ops():
    """Register the three fused single-port DVE ops (idempotent)."""
    import concourse.dve_ops as dve_ops
    from concourse.dve_spec import (
        Spec, Src0, Zero, One, select, lower, AluOp, Bin, scan, C0, C2,
    )
    from concourse.dve_uop import DveOpSpec

    def _get(name):
        for op in dve_ops.OPS:
            if op.name == name:
                return op
        return None

    def _register(name, spec):
        existing = _get(name)
        if existing is not None:
            return existing
        opcode = dve_ops._CUSTOM_DVE_ROW_BASE + len(dve_ops.OPS)
        assert opcode < 0x20, "custom DVE opcode rows exhausted"
        from concourse.dve_ops import has_src1
        shas = {}
        for ver in ("v3", "v4"):
            uops = lower(spec, ver=ver)
            tmp = DveOpSpec(name=name, opcode=opcode, uops=uops,
                            rd1_en=has_src1(spec))
            shas[ver] = tmp.sha(ver)
        op = dve_ops.DveOp(name, spec, subdim=False, uops_sha=shas)
        dve_ops.OPS.append(op)
        dve_ops._SUB_OPCODE_FOR_NAME[name] = opcode
        dve_ops.CUSTOM_DVE_SPECS[name] = spec
        return op

    # GATHER_SCAN: hold = running max; keep x where hold-(imm2+k) in [0,s0)
    def _gather_ref(in0, in1, s0, s1, imm2):
        p = in0.shape[0]
        x = np.asarray(in0, np.float32).reshape(p, -1)
        hold = np.maximum.accumulate(x, axis=1)
        idxk = np.float32(imm2) + np.arange(x.shape[1],
                                            dtype=np.float32)[None, :]
        diff = hold - idxk
        s0v = np.asarray(s0, np.float32).reshape(p, 1) \
            if isinstance(s0, np.ndarray) else np.float32(s0)
        keep = (diff >= 0) & (diff < s0v)
        out = np.where(keep, x, np.float32(0.0))
        acc = out.sum(axis=1, dtype=np.float64).astype(np.float32)[:, None]
        return out.reshape(in0.shape), acc

    idxk = scan(AluOp.ADD, One, init=Bin(AluOp.SUBTRACT, C2, One))
    hold = scan(AluOp.MAX, Src0)
    diff = hold - idxk
    gather_spec = Spec(
        body=select((diff >= Zero) & (diff < C0), Src0, Zero),
        accum=AluOp.ADD,
        accum_init=Zero,
        reference=_gather_ref,
    )
    gop = _register("GATHER_SCAN_ANT", gather_spec)

    # SB_EXTRACT: sb = 1 - 2*(frac(x) > 0), frac via the +2^23 round trick
    def _sb_ref(in0, in1, s0, s1, imm2):
        p = in0.shape[0]
        x = np.asarray(in0, np.float32).reshape(p, -1)
        r = (x + np.float32(imm2)).astype(np.float32) - np.float32(imm2)
        fr = x - r.astype(np.float32)
        sb = np.where(fr > 0, np.float32(-1.0), np.float32(1.0))
        return sb.reshape(in0.shape)

    r = (Src0 + C2) - C2
    g = (Src0 - r) > Zero
    sb_spec = Spec(
        body=(One - g) - g,
        reference=_sb_ref,
    )
    sop = _register("SB_EXTRACT_ANT", sb_spec)

    # SUM_GT: accum += (x > 0)
    def _gt_ref(in0, in1, s0, s1, imm2):
        p = in0.shape[0]
        x = np.asarray(in0, np.float32).reshape(p, -1)
        out = (x > 0).astype(np.float32)
        acc = out.sum(axis=1, dtype=np.float64).astype(np.float32)[:, None]
        return out.reshape(in0.shape), acc

    gt_spec = Spec(
        body=Src0 > Zero,
        accum=AluOp.ADD,
        accum_init=Zero,
        reference=_gt_ref,
    )
    qop = _register("SUM_GT_ANT", gt_spec)
    return gop, sop, qop


# ------------------------------------------------------------- device build
def _build_nc(tile_ws=None):
    import concourse.bass as bass
    import concourse.tile as tile
    from concourse import bacc, mybir

    gop, sop, qop = _register_custom_ops()
    f32 = mybir.dt.float32
    i32 = mybir.dt.int32
    bf16 = mybir.dt.bfloat16
    A = mybir.ActivationFunctionType
    X = mybir.AxisListType.X
    XY = mybir.AxisListType.XY
    alu = mybir.AluOpType

    if tile_ws is None:
        tile_ws = TILE_WS
    tiles = len(tile_ws)
    nc = bacc.Bacc("TRN2", target_bir_lowering=False, debug=False,
                   num_devices=N_CORES)
    comb_ds = [
        nc.dram_tensor(f"comb{i}", [P, wi * CW], f32,
                       kind="ExternalInput").ap()
        for i, wi in enumerate(tile_ws)
    ]
    out_d = nc.dram_tensor("out", [P, 3], f32, kind="ExternalOutput").ap()

    with tile.TileContext(nc) as tc:
        with (
            tc.tile_pool(name="io", bufs=3) as io,
            tc.tile_pool(name="ep", bufs=3) as ep,
            tc.tile_pool(name="zp", bufs=2) as zp,
            tc.tile_pool(name="mp", bufs=2) as mp,
            tc.tile_pool(name="cp", bufs=1) as cp,
        ):
            bias16 = cp.tile([P, 1], f32)
            nc.gpsimd.memset(bias16[:], -SHIFT)
            acc_all = cp.tile([P, 3, tiles], f32)
            acc_lg = acc_all[:, 0, :]
            acc_g = acc_all[:, 1, :]
            acc_mm = acc_all[:, 2, :]

            for i in range(tiles):
                w = tile_ws[i]
                ct = io.tile([P, w * CW], f32, tag="comb")
                nc.sync.dma_start(ct[:], comb_ds[i])
                cv = ct[:].rearrange("p (w x) -> p w x", x=CW)
                v10 = cv[:, :, 1:CW]

                # ---- CE path: exp on ACT, row-sum tree on GPSIMD, ln on ACT
                et = ep.tile([P, w * C], bf16, tag="E")
                nc.scalar.activation(et[:], v10, A.Exp, bias=bias16[:])

                e3 = et[:].rearrange("p (w c) -> p w c", c=C)
                z1 = zp.tile([P, w, 5], bf16, tag="z1")
                nc.gpsimd.tensor_tensor(z1[:], e3[:, :, 0:5], e3[:, :, 5:10],
                                        op=alu.add)
                z2 = zp.tile([P, w, 2], bf16, tag="z2")
                nc.gpsimd.tensor_tensor(z2[:], z1[:, :, 0:2], z1[:, :, 2:4],
                                        op=alu.add)
                z3 = zp.tile([P, w], bf16, tag="z3")
                nc.gpsimd.tensor_tensor(z3[:], z2[:, :, 0], z2[:, :, 1],
                                        op=alu.add)
                s = zp.tile([P, w], bf16, tag="s")
                nc.gpsimd.tensor_tensor(s[:], z3[:], z1[:, :, 4], op=alu.add)

                lg = zp.tile([P, w], f32, tag="lg")
                nc.scalar.activation(lg[:], s[:], A.Ln,
                                     accum_out=acc_lg[:, i:i + 1])

                # ---- BCE path: group max reduces on DVE (1 port each)
                m6 = mp.tile([P, w], f32, tag="m6")
                nc.vector.reduce_max(m6[:], cv[:, :, 3:9], axis=X)
                v4 = v10.rearrange("p w (g e) -> p w g e", g=5, e=2)
                m4n = mp.tile([P, w], f32, tag="m4n")
                nc.vector.reduce_max(m4n[:], v4[:, :, 0:5:4, :], axis=XY,
                                     negate=True)
                sb = mp.tile([P, w], f32, tag="sb")
                nc.vector._custom_dve(
                    sop, out=sb[:], in0=cv[:, :, 0], imm2=8388608.0)

                # d6 = m6 - m4, q = d6 * sb on GPSIMD (add/mult only)
                d6 = mp.tile([P, w], f32, tag="d6")
                nc.gpsimd.tensor_tensor(d6[:], m6[:], m4n[:], op=alu.add)
                q = mp.tile([P, w], f32, tag="q")
                nc.gpsimd.tensor_tensor(q[:], d6[:], sb[:], op=alu.mult)

                # ---- fused single-port gather + mismatch count (DVE)
                nc.vector._custom_dve(
                    gop, out=ct[:], in0=ct[:], s0=0.5, imm2=HBASE,
                    accum_out=acc_g[:, i:i + 1])
                nc.vector._custom_dve(
                    qop, out=q[:], in0=q[:],
                    accum_out=acc_mm[:, i:i + 1])

            # ---- final per-partition reduction + store (one fused reduce)
            out_t = cp.tile([P, 3], f32)
            nc.vector.reduce_sum(out_t[:], acc_all[:], axis=X)
            nc.sync.dma_start(out_d[:], out_t[:])

    # Force a single activation table containing both Exp and Ln so the
    # compiler does not ping-pong ACT_TABLE_LOADs.  Table ids are positional,
    # so keep the dict shape and empty the other sets.
    import concourse.bacc as bacc_mod
    from concourse.hw_specs import get_activation_tables
    orig = get_activation_tables(nc.m.arch)
    combined = None
    for k, v in orig.items():
        if (mybir.ActivationFunctionType.Exp in v
                and mybir.ActivationFunctionType.Ln in v):
            combined = k
            break
    if combined is not None:
        patched = {k: (v if k == combined else set()) for k, v in orig.items()}
        saved = bacc_mod.get_activation_tables
        bacc_mod.get_activation_tables = lambda arch: patched
        try:
            nc.compile()
        finally:
            bacc_mod.get_activation_tables = saved
    else:
        nc.compile()
    return nc


def _get_nc():
    if "nc" not in _CACHE:
        _CACHE["nc"] = _build_nc()
    return _CACHE["nc"]


# ------------------------------------------------------------------- host
def _host_prep(pred, target):
    """Shard + pad inputs, pack [h, pred+16] rows per core."""
    pred = np.asarray(pred, dtype=np.float32)
    target = np.asarray(target).astype(np.int32)

    in_maps = []
    rows = ROWS_CORE
    for c in range(N_CORES):
        pc = pred[c * rows:(c + 1) * rows] + np.float32(SHIFT)
        tc_ = target[c * rows:(c + 1) * rows]
        if PAD_PER_CORE:
            pc = np.concatenate(
                [pc, np.full((PAD_PER_CORE, C), SHIFT, np.float32)], axis=0)
            tc_ = np.concatenate(
                [tc_, np.zeros(PAD_PER_CORE, np.int32)], axis=0)
        m = {}
        off = 0
        for i, wi in enumerate(TILE_WS):
            n_i = P * wi
            pi = pc[off:off + n_i].reshape(P, wi, C)
            ti = tc_[off:off + n_i].reshape(P, wi)
            off += n_i
            bt = (ti >= 2) & (ti <= 7)
            w_idx = np.arange(wi, dtype=np.float64)[None, :] * CW
            h = (HBASE + w_idx + 1.0 + ti + 0.25 * bt).astype(np.float32)
            comb = np.empty((P, wi, CW), np.float32)
            comb[..., 0] = h
            comb[..., 1:] = pi
            m[f"comb{i}"] = np.ascontiguousarray(comb.reshape(P, wi * CW))
        in_maps.append(m)
    return in_maps


def kernel(pred, target):
    from concourse.bass_utils import run_bass_kernel_spmd

    nc = _get_nc()
    in_maps = _host_prep(pred, target)
    res = run_bass_kernel_spmd(nc, in_maps, core_ids=list(range(N_CORES)))

    sum_lg = 0.0
    sum_g = 0.0
    sum_mm = 0.0
    for c in range(N_CORES):
        o = res.results[c]["out"].astype(np.float64)
        sum_lg += o[:, 0].sum()
        sum_g += o[:, 1].sum()
        sum_mm += o[:, 2].sum()

    # padded rows: pred' = 16 -> logsumexp = ln(10), gather = 16, mismatch 0.
    # every (real and padded) row's gather picks pred + 16.
    sum_lg -= N_CORES * PAD_PER_CORE * np.log(10.0)
    sum_g -= SHIFT * N_CORES * ROWS_CORE_PAD

    ce = (sum_lg - sum_g) / N
    bce = 100.0 * sum_mm / N
    return np.float32(ce + bce)



# revision 7
# speedup vs baseline: 1.3511x; 1.3511x over previous
"""Trainium2 Bass kernel for nn_CustomLoss (CrossEntropy + binary-remap BCE).

loss = mean_i[ logsumexp(pred_i) - pred_i[t_i] ]
     + 100 * mean_i[ 1{ LUT[argmax(pred_i)] != LUT[t_i] } ]

with LUT = [0,0,1,1,1,1,1,1,0,0]  (LUT[j] = 1 iff 2 <= j <= 7).

Data-parallel over the batch across 8 NeuronCores.  The host re-packs each
row so the device needs neither a gather nor a sign trick:

  * rows are partitioned into region A (binary target bt=0, t in {0,1,8,9})
    and region B (bt=1, t in 2..7);
  * each row's 10 logits are permuted to  [X-group | pred[t] | Y-group-rest]
    where X = the class group NOT containing t and Y = the group containing
    t.  Then for every row, mismatch == (max X > max Y), and pred[t] sits at
    a fixed slot (6 in region A, 4 in region B);
  * everything ships as bf16 (22B/row less DMA than f32), +16 pre-shift so
    exp(x-16) = exp(pred).

Device per tile (region-typed widths, compile-time slot bounds):
  ACT   : E = exp(row - 16) (one flat bf16 instr), Ln(rowsum) with
          per-partition accumulate.  Single Exp+Ln table (no reloads).
  DVE   : packed-bf16 tensor_tensor add/max trees (2-byte packed operands
          hit the 2x/4x DVE fast modes), strided slice reduce for sum of
          pred[t], and one scalar_tensor_tensor is_gt with accum_out for
          the mismatch count.
  GPSIMD: the two strided tree tails (cheap, keeps DVE free).
"""

import numpy as np
import ml_dtypes

# ---------------------------------------------------------------- constants
N = 2_000_000
C = 10
N_CORES = 8
P = 128
ROWS_CORE = N // N_CORES      # 250,000
SHIFT = 16.0
MID = (2, 3, 4, 5, 6, 7)      # classes with LUT == 1
OUTER = (0, 1, 8, 9)          # classes with LUT == 0

# slot permutation per target class: [X-group | t | Y-group minus t]
_IDX_TAB = np.empty((C, C), np.int64)
for _t in range(C):
    if _t in OUTER:   # region A: X = mid(6), Y = t + outer\{t}(3)
        _IDX_TAB[_t] = list(MID) + [_t] + [c for c in OUTER if c != _t]
    else:             # region B: X = outer(4), Y = t + mid\{t}(5)
        _IDX_TAB[_t] = list(OUTER) + [_t] + [c for c in MID if c != _t]

_CACHE = {}


def _split_w(w, first=110, body=352):
    """Tile widths: small first tile to prime the pipeline."""
    ws = []
    if w > first + 64:
        ws.append(first)
        w -= first
    while w > 0:
        c = min(body, w)
        if 0 < w - c < 64:
            c = w          # avoid a tiny trailing tile
        ws.append(c)
        w -= c
    return ws


# ------------------------------------------------------------- device build
def _build_nc(tiles_a, tiles_b):
    import concourse.bass as bass  # noqa: F401  (env setup)
    import concourse.tile as tile
    from concourse import bacc, mybir

    f32 = mybir.dt.float32
    bf16 = mybir.dt.bfloat16
    A = mybir.ActivationFunctionType
    X = mybir.AxisListType.X
    alu = mybir.AluOpType

    tiles = [("a", w) for w in tiles_a] + [("b", w) for w in tiles_b]
    T = len(tiles)

    nc = bacc.Bacc("TRN2", target_bir_lowering=False, debug=False,
                   num_devices=N_CORES)
    comb_ds = [
        nc.dram_tensor(f"comb{i}", [P, w * C], bf16, kind="ExternalInput").ap()
        for i, (_, w) in enumerate(tiles)
    ]
    out_d = nc.dram_tensor("out", [P, 4], f32, kind="ExternalOutput").ap()

    with tile.TileContext(nc) as tc:
        with (
            tc.tile_pool(name="io", bufs=3) as io,
            tc.tile_pool(name="ep", bufs=3) as ep,
            tc.tile_pool(name="zp", bufs=3) as zp,
            tc.tile_pool(name="mp", bufs=3) as mp,
            tc.tile_pool(name="cp", bufs=1) as cp,
        ):
            bias16 = cp.tile([P, 1], f32)
            nc.gpsimd.memset(bias16[:], -SHIFT)
            acc_all = cp.tile([P, 4, T], f32)
            acc_lg = acc_all[:, 0, :]
            acc_g = acc_all[:, 1, :]
            acc_mm = acc_all[:, 2, :]
            acc_me = acc_all[:, 3, :]

            for i, (reg, w) in enumerate(tiles):
                ct = io.tile([P, w * C], bf16, tag="comb")
                nc.sync.dma_start(ct[:], comb_ds[i])
                cv = ct[:].rearrange("p (w s) -> p w s", s=C)

                # ---- CE: exp (flat), packed add tree, ln with accumulate
                et = ep.tile([P, w * C], bf16, tag="E")
                nc.scalar.activation(et[:], ct[:], A.Exp, bias=bias16[:])
                ev = et[:].rearrange("p (w s) -> p w s", s=C)

                z = zp.tile([P, w, 5], bf16, tag="z")
                nc.vector.tensor_tensor(z[:], ev[:, :, 0:5], ev[:, :, 5:10],
                                        op=alu.add)
                cp2 = zp.tile([P, w, 2], bf16, tag="c")
                nc.vector.tensor_tensor(cp2[:], z[:, :, 0:2], z[:, :, 2:4],
                                        op=alu.add)
                d = zp.tile([P, w], bf16, tag="d")
                nc.gpsimd.tensor_tensor(d[:], cp2[:, :, 0], cp2[:, :, 1],
                                        op=alu.add)
                s = zp.tile([P, w], bf16, tag="s")
                nc.gpsimd.tensor_tensor(s[:], d[:], z[:, :, 4], op=alu.add)
                lg = zp.tile([P, w], f32, tag="lg")
                nc.scalar.activation(lg[:], s[:], A.Ln,
                                     accum_out=acc_lg[:, i:i + 1])

                # ---- sum of pred'[t]: fixed slot per region
                u = 6 if reg == "a" else 4
                nc.vector.reduce_sum(acc_g[:, i:i + 1], cv[:, :, u], axis=X)

                # ---- BCE: packed max trees, then one is_gt with accum
                if reg == "a":
                    m1 = mp.tile([P, w, 2], bf16, tag="m1")
                    nc.vector.tensor_tensor(m1[:], cv[:, :, 0:2],
                                            cv[:, :, 2:4], op=alu.max)
                    m2 = mp.tile([P, w, 2], bf16, tag="m2")
                    nc.vector.tensor_tensor(m2[:], m1[:], cv[:, :, 4:6],
                                            op=alu.max)
                    mx = mp.tile([P, w], bf16, tag="mx")
                    nc.vector.tensor_tensor(mx[:], m2[:, :, 0], m2[:, :, 1],
                                            op=alu.max)
                    n1 = mp.tile([P, w, 2], bf16, tag="n1")
                    nc.vector.tensor_tensor(n1[:], cv[:, :, 6:8],
                                            cv[:, :, 8:10], op=alu.max)
                    my = mp.tile([P, w], bf16, tag="my")
                    nc.vector.tensor_tensor(my[:], n1[:, :, 0], n1[:, :, 1],
                                            op=alu.max)
                else:
                    m1 = mp.tile([P, w, 2], bf16, tag="m1")
                    nc.vector.tensor_tensor(m1[:], cv[:, :, 0:2],
                                            cv[:, :, 2:4], op=alu.max)
                    mx = mp.tile([P, w], bf16, tag="mx")
                    nc.vector.tensor_tensor(mx[:], m1[:, :, 0], m1[:, :, 1],
                                            op=alu.max)
                    n1 = mp.tile([P, w, 2], bf16, tag="n1")
                    nc.vector.tensor_tensor(n1[:], cv[:, :, 4:6],
                                            cv[:, :, 6:8], op=alu.max)
                    n2 = mp.tile([P, w, 2], bf16, tag="n2")
                    nc.vector.tensor_tensor(n2[:], n1[:], cv[:, :, 8:10],
                                            op=alu.max)
                    my = mp.tile([P, w], bf16, tag="my")
                    nc.vector.tensor_tensor(my[:], n2[:, :, 0], n2[:, :, 1],
                                            op=alu.max)
                # bf16 rounding is monotone, so mx/my are exactly the rounded
                # group maxes; ties (equal after rounding) are true-greater
                # with probability ~1/2.  Count (is_gt + is_ge)/2.
                q = mp.tile([P, w], bf16, tag="q")
                nc.vector.scalar_tensor_tensor(
                    q[:], mx[:], 0.0, my[:], op0=alu.add, op1=alu.is_gt,
                    accum_out=acc_mm[:, i:i + 1])
                q2 = mp.tile([P, w], bf16, tag="q2")
                nc.vector.scalar_tensor_tensor(
                    q2[:], mx[:], 0.0, my[:], op0=alu.add, op1=alu.is_ge,
                    accum_out=acc_me[:, i:i + 1])

            # ---- final per-partition fold + store
            out_t = cp.tile([P, 4], f32)
            nc.vector.reduce_sum(out_t[:], acc_all[:], axis=X)
            nc.sync.dma_start(out_d[:], out_t[:])

    # Single activation table containing both Exp and Ln so the compiler
    # does not ping-pong ACT_TABLE_LOADs.
    import concourse.bacc as bacc_mod
    from concourse.hw_specs import get_activation_tables
    orig = get_activation_tables(nc.m.arch)
    combined = None
    for k, v in orig.items():
        if (mybir.ActivationFunctionType.Exp in v
                and mybir.ActivationFunctionType.Ln in v):
            combined = k
            break
    if combined is not None:
        patched = {k: (v if k == combined else set()) for k, v in orig.items()}
        saved = bacc_mod.get_activation_tables
        bacc_mod.get_activation_tables = lambda arch: patched
        try:
            nc.compile()
        finally:
            bacc_mod.get_activation_tables = saved
    else:
        nc.compile()
    return nc


def _get_nc():
    key = _CACHE["tiles_key"]
    if ("nc", key) not in _CACHE:
        _CACHE[("nc", key)] = _build_nc(*key)
    return _CACHE[("nc", key)]


# ------------------------------------------------------------------- host
def _host_prep(pred, target):
    """Shard, region-sort, permute slots, pad, tile. Returns in_maps and
    stores layout info in _CACHE."""
    pred = np.asarray(pred, dtype=np.float32)
    target = np.asarray(target).astype(np.int64)

    predp = (pred + np.float32(SHIFT)).astype(ml_dtypes.bfloat16)
    colidx = _IDX_TAB[target]                      # [N, 10]
    packed = np.take_along_axis(predp, colidx, axis=1)
    bt = ((target >= 2) & (target <= 7))

    # per-core region rows
    core_a, core_b = [], []
    for c in range(N_CORES):
        sl = slice(c * ROWS_CORE, (c + 1) * ROWS_CORE)
        btc = bt[sl]
        pc = packed[sl]
        core_a.append(pc[~btc])
        core_b.append(pc[btc])

    wa = max((a.shape[0] + P - 1) // P for a in core_a)
    wb = max((b.shape[0] + P - 1) // P for b in core_b)
    tiles_a = _split_w(wa)
    tiles_b = _split_w(wb)
    _CACHE["tiles_key"] = (tuple(tiles_a), tuple(tiles_b))

    n_pad = 0
    in_maps = []
    for c in range(N_CORES):
        m = {}
        i = 0
        for rows_all, w_tot, tiles in ((core_a[c], wa, tiles_a),
                                       (core_b[c], wb, tiles_b)):
            n = rows_all.shape[0]
            pad = P * w_tot - n
            n_pad += pad
            if pad:
                rows_all = np.concatenate(
                    [rows_all, np.zeros((pad, C), ml_dtypes.bfloat16)], axis=0)
            # [P*w, 10] -> [P, w, 10] partition-major
            rows_all = rows_all.reshape(P, w_tot, C)
            off = 0
            for w in tiles:
                m[f"comb{i}"] = np.ascontiguousarray(
                    rows_all[:, off:off + w, :].reshape(P, w * C))
                off += w
                i += 1
        in_maps.append(m)
    _CACHE["n_pad"] = n_pad
    return in_maps


def _pad_ln_const():
    """ln(S) the device computes for an all-zero pad row, replicated in
    bf16 arithmetic: E = exp(-16), tree 10 -> 5x2 -> 2x2 -> ... """
    bf = ml_dtypes.bfloat16
    e = bf(np.exp(np.float32(-SHIFT)))
    z = bf(np.float32(e) + np.float32(e))        # each of z0..z4
    c2 = bf(np.float32(z) + np.float32(z))       # cpair
    d = bf(np.float32(c2) + np.float32(c2))
    s = bf(np.float32(d) + np.float32(z))
    return float(np.log(np.float64(s)))


def kernel(pred, target):
    from concourse.bass_utils import run_bass_kernel_spmd

    in_maps = _host_prep(pred, target)
    nc = _get_nc()
    res = run_bass_kernel_spmd(nc, in_maps, core_ids=list(range(N_CORES)))

    sum_lg = 0.0
    sum_g = 0.0
    sum_mm = 0.0
    for c in range(N_CORES):
        o = res.results[c]["out"].astype(np.float64)
        sum_lg += o[:, 0].sum()
        sum_g += o[:, 1].sum()
        sum_mm += 0.5 * (o[:, 2].sum() + o[:, 3].sum())

    # pad rows: lnS = pad const, pred[t] slot = 0, no mismatch
    sum_lg -= _CACHE["n_pad"] * _pad_ln_const()
    ce = (sum_lg - sum_g) / N + SHIFT
    bce = 100.0 * sum_mm / N
    return np.float32(ce + bce)
